# revision 57
# baseline (speedup 1.0000x reference)
"""Trainium2 Bass kernel for nn_BindingSiteGraphSAGE (3-layer GraphSAGE + MLP head).

Self-contained: takes the full inputs, shards destination nodes across the
8 NeuronCores, runs a single SPMD Bass program (edge aggregation via
indirect-DMA gathers + weighted-onehot PSUM matmuls, with per-superblock
interleaved dense phases and per-superblock AllGather collectives between
layers), and returns the full [50000, 2] float32 output.

Optimizations over the first working version:
- all dense matmuls run in bf16 (fp32 costs 4 cycles/row on the PE).
- onehot builds use a bf16 iota so the DVE runs in its 2x perf mode.
- nodes are relabeled host-side (balance_nodes) so every (core, superblock,
  half) has a near-equal edge count: the SPMD schedule fetches the
  max-over-cores count per gather, so assignment skew is pure wasted DMA.
- each (superblock, half) packs its edges as ONE dense slot stream
  (block-major); gathers split it at tile boundaries, so only the final
  chunk of a stream carries (trimmed) padding. Tiles straddling a block
  boundary get one onehot+matmul per (tile, block) "piece".
- num_idxs trimming skips tail pad slots; skipped slots are matmul'ed with
  onehot zeros, which is safe because their SBUF bytes hold finite values
  from a previous chunk (a buffer's first use at a new byte extent is
  fetched untrimmed instead of memset).
- leaky ReLU is a single Prelu activation (alpha=0.15).
- pre_fc has no activation, so Wp/bp fuse into Wf1/bf1 host-side (one
  matmul + one activation fewer per head chain, incl. the program tail).
- the L2/L3 self terms stay SBUF-resident and are folded into the psum by
  an identity matmul on the PE (drains become a single Act Prelu).
- gather tables (x copy, t2, t3) share one superblock-major row layout
  ("pid" space): each superblock's AllGather output is contiguous, so the
  AllGathers run per superblock as soon as that superblock's table rows are
  computed, overlapping the remaining compute; only the last small chunk's
  latency is exposed. One gather-index table serves all three layers.
- the [2, N] output is staged in SBUF and written to DRAM once at the end
  (transposed so the write is two contiguous descriptors, not 8B rows).

Host preprocessing only reorders/pads the edge list and ships structural
metadata (indices, degree weights, schedules, a row-permuted copy of x) —
no feature math on host.
"""
import sys
for _p in ("/opt/trn_rl_repo",):
    if _p not in sys.path:
        sys.path.insert(0, _p)
import numpy as np
import concourse.bacc as bacc
import concourse.tile as tile
import concourse.mybir as mybir
from concourse.bass_utils import run_bass_kernel_spmd

"""Host-side graph structure preprocessing.

Partitions dst nodes across cores, builds a uniform (core-independent)
tile schedule for edge aggregation, and per-core gather/onehot buffers.

Layout conventions:
- dst slice per core: cfg['slice'] real nodes, padded to cfg['slice_pad'];
  node -> (core, slot) comes from balance_nodes, not node-id arithmetic.
- dst blocks of W=128 local dsts; superblocks per cfg['sb_sizes'].
- gather-table rows live in "pid" space, superblock-major:
  pid = 4096*sb + core*sbc(sb) + (slot - 512*sb); superblocks 0-7
  occupy pids [0, 32768) ("lo"), 8-12 occupy [32768, 50176) ("hi"), so
  int16 gather indices are pid or pid - 32768.
- edges split lo/hi by the src's slot (slot < half_r).
- per (sb, half): one dense block-major slot stream; tile count = max over
  cores; chunks of <= MAX_CHUNK_TILES tiles; pieces per (tile, block).
- gather idx i -> partition i%16, free i//16 (replicated x8 over partitions).
- seg per (piece, slot): partition i%128, free piece index (seg 128 =
  pad / slot not in this piece's block); wgt per (tile, slot) -- shared by
  the tile's pieces, since masked slots zero out in is_equal anyway.
"""

W = 128


def make_cfg(n_cores, n_nodes, slice_, slice_pad, sb_sizes):
    assert slice_pad % W == 0 and sum(sb_sizes) == slice_pad // W
    assert sb_sizes == [4] * 12 + [1], "pid map hardcodes this layout"
    return dict(n_cores=n_cores, n_nodes=n_nodes, slice=slice_,
                slice_pad=slice_pad, sb_sizes=sb_sizes,
                half_r=4096, hi_base=32768, n_lo_sb=8,
                pid_n=n_cores * slice_pad)


CFG_FULL = make_cfg(8, 50000, 6250, 6272, [4] * 12 + [1])


def balance_nodes(edge_index, cfg):
    """Assign nodes to (core, slot) so per-(core, superblock, half) edge
    counts are near-equal across cores (the SPMD schedule fetches the
    max-over-cores count per gather stream, so skew is pure wasted DMA).

    The first 32768 node ids form the "lo set" and must land in slots
    < half_r (their outgoing edges classify as lo at the src side). Within
    that constraint, spread nodes type-by-type into (core, superblock) bins,
    leveling the per-bin (lo, hi) in-degree sums.

    Returns (node2core, node2slot) int32 arrays of length n_nodes.
    """
    src = np.asarray(edge_index[0], np.int64)
    dst = np.asarray(edge_index[1], np.int64)
    NN, C, SP = cfg["n_nodes"], cfg["n_cores"], cfg["slice_pad"]
    HALF_R = cfg["half_r"]
    NB = SP // W                       # blocks per core (49)
    LO_NODES = C * HALF_R              # 32768
    lo_src = src < LO_NODES
    deg_lo = np.bincount(dst[lo_src], minlength=NN).astype(np.int64)
    deg_hi = np.bincount(dst[~lo_src], minlength=NN).astype(np.int64)

    n2c = np.empty(NN, np.int32)
    n2s = np.empty(NN, np.int32)

    def assign(nodes, blocks, caps, slot0):
        """Place `nodes` into bins (core, b) for b in `blocks`; caps[b] = slots
        per bin; slot0[b] = first slot index of block b.

        Nodes with identical (deg_lo, deg_hi) are interchangeable, so work on
        types: spread each type's count evenly over all bins, steering the
        remainder to the bins currently furthest below their (lo, hi) targets.
        """
        bins_c = np.repeat(np.arange(C), len(blocks))
        bins_b = np.tile(np.array(blocks), C)
        NBIN = len(bins_c)
        cap = np.array([caps[b] for b in bins_b], np.int64)
        tl = max(float(deg_lo[nodes].sum()), 1.0) / NBIN
        th = max(float(deg_hi[nodes].sum()), 1.0) / NBIN
        slo = np.zeros(NBIN); shi = np.zeros(NBIN)
        cnt = np.zeros(NBIN, np.int64)
        dl_n, dh_n = deg_lo[nodes], deg_hi[nodes]
        # types sorted by descending total degree
        tkey = dl_n * 1000 + dh_n
        uniq, inv = np.unique(tkey, return_inverse=True)
        order = np.argsort(-uniq)
        for t in order:
            members = nodes[inv == t]
            dl, dh = int(uniq[t]) // 1000, int(uniq[t]) % 1000
            m = len(members)
            take = np.zeros(NBIN, np.int64)
            rem = m
            while rem > 0:
                room = cap - cnt - take
                navail = int((room > 0).sum())
                assert navail > 0
                per = rem // navail
                if per > 0:
                    add = np.minimum(room, per)
                    take += add
                    rem -= int(add.sum())
                else:
                    # rank bins by how far below target they'd be after
                    # adding, weighting each dimension by this type's
                    # contribution (a dh=0 type should level slo only)
                    score = (dl * (slo + take * dl + dl) / tl
                             + dh * (shi + take * dh + dh) / th)
                    score[room <= 0] = np.inf
                    extra = np.argsort(score, kind="stable")[:rem]
                    take[extra] += 1
                    rem = 0
            pos = 0
            for i in np.flatnonzero(take):
                k = int(take[i])
                sel = members[pos:pos + k]
                pos += k
                n2c[sel] = bins_c[i]
                n2s[sel] = slot0[bins_b[i]] + cnt[i] + np.arange(k)
                cnt[i] += k
                slo[i] += k * dl
                shi[i] += k * dh
            assert pos == m
        assert (cnt == cap).all()

    # bins are (core, superblock): the gather schedule's cost is set by the
    # max-over-cores edge count per (superblock, half), so balance at that
    # granularity (512-node bins balance much tighter than 128-node blocks).
    NSB = 13
    slot0 = {s: s * 512 for s in range(NSB)}
    lo_nodes = np.arange(LO_NODES)
    hi_nodes = np.arange(LO_NODES, NN)
    assign(lo_nodes, list(range(8)), {s: 512 for s in range(8)}, slot0)
    hi_caps = {s: 512 for s in range(8, NSB)}
    hi_caps[NSB - 1] = (NN // C) - 12 * 512      # last superblock: 106 real
    assign(hi_nodes, list(range(8, NSB)), hi_caps, slot0)
    return n2c, n2s

# a superblock-half's edges are packed as one dense slot stream (block-major);
# gathers split the stream at tile boundaries into chunks of at most this many
# tiles. Only the final chunk of each stream carries padding (trimmed via
# num_idxs), so mid-stream chunk splits cost nothing.
MAX_CHUNK_TILES = 11
GATHER_BUFS = 12


def pid_of_slot(c, r, cfg):
    """(core, local slot) -> row in the superblock-major gather tables."""
    r = np.asarray(r, dtype=np.int64)
    s = np.minimum(r // 512, 12)       # 6144..6271 -> 12 (last, short sb)
    sbc = np.where(s < 12, 512, 128)
    return 4096 * s + c * sbc + (r - 512 * s)


def build_structure(edge_index, cfg):
    src = np.asarray(edge_index[0], dtype=np.int64)
    dst = np.asarray(edge_index[1], dtype=np.int64)
    C, SL, SP = cfg["n_cores"], cfg["slice"], cfg["slice_pad"]
    HALF_R, HI_BASE = cfg["half_r"], cfg["hi_base"]

    n2c, n2s = balance_nodes(edge_index, cfg)
    pid_tab = np.empty(cfg["n_nodes"], np.int64)
    for c in range(C):
        m = n2c == c
        pid_tab[m] = pid_of_slot(c, n2s[m], cfg)

    percore_edges = []
    for c in range(C):
        m = n2c[dst] == c
        es, ed = src[m], n2s[dst[m]].astype(np.int64)
        cnt = np.bincount(ed, minlength=SP).astype(np.float32)
        wall = 1.0 / np.maximum(cnt, 1.0)
        percore_edges.append((es, ed, wall))

    sched = []
    percore = [dict(idx=[], seg=[], wgt=[]) for _ in range(C)]
    hlo_all = [n2s[percore_edges[c][0]] < HALF_R for c in range(C)]
    b0 = 0
    for sb_sz in cfg["sb_sizes"]:
        blocks = list(range(b0, b0 + sb_sz))
        col_lo, col_hi = b0 * W, (b0 + sb_sz) * W
        ntiles = {}
        nidx_h = {}
        pieces = {}          # (h, t) -> local block indices present (any core)
        streams = {}         # (h, c) -> (pids, scol, wgts) block-major dense
        for h in (0, 1):
            me = 0
            for c in range(C):
                es, ed, wall = percore_edges[c]
                m = (ed >= col_lo) & (ed < col_hi) & \
                    (hlo_all[c] if h == 0 else ~hlo_all[c])
                es_m, ed_m = es[m], ed[m]
                o = np.argsort(ed_m, kind="stable")
                es_m, ed_m = es_m[o], ed_m[o]
                streams[(h, c)] = (pid_tab[es_m], ed_m - col_lo, wall[ed_m])
                me = max(me, len(es_m))
            nt = max(1, ceil_div(me, W))
            ntiles[h] = nt
            # exact trim: the cost model bills num_idxs descriptors verbatim
            nidx_h[h] = min(nt * W, max(me, (nt - 1) * W + 1))
            for t in range(nt):
                bls = set()
                for c in range(C):
                    seg = streams[(h, c)][1][t * W:(t + 1) * W]
                    if len(seg):
                        bls.update((seg // W).tolist())
                pieces[(h, t)] = sorted(bls) or [0]

        tiles = [(h, t) for h in (0, 1) for t in range(ntiles[h])]
        chunks = []
        tbase = 0
        for h in (0, 1):
            nt = ntiles[h]
            parts = ceil_div(nt, MAX_CHUNK_TILES)
            t0 = 0
            for pi in range(parts):
                t1 = t0 + nt // parts + (1 if pi < nt % parts else 0)
                nidx = (t1 - t0) * W if t1 < nt else nidx_h[h] - t0 * W
                chunks.append((h, tbase + t0, tbase + t1, nidx))
                t0 = t1
            assert t0 == nt
            tbase += nt

        for c in range(C):
            for h in (0, 1):
                pids, scol, wgts = streams[(h, c)]
                nt = ntiles[h]
                ne = len(pids)
                assert ne <= nt * W
                ix = np.zeros(nt * W, np.int16)
                pv = pids if h == 0 else pids - HI_BASE
                assert ne == 0 or (pv.min() >= 0 and pv.max() < 32768)
                ix[:ne] = pv.astype(np.int16)
                percore[c]["idx"].append(ix)
                for t in range(nt):
                    s_t = scol[t * W:(t + 1) * W]
                    w_t = wgts[t * W:(t + 1) * W]
                    ns = len(s_t)
                    # one unmasked weight column per tile: slots outside a
                    # piece's block give all-zero is_equal rows, so their
                    # weight value never matters
                    wg = np.zeros(W, np.float32)
                    wg[:ns] = w_t
                    percore[c]["wgt"].append(wg)
                    for bl in pieces[(h, t)]:
                        sg = np.full(W, W, np.float32)
                        if ns:
                            mblk = (s_t // W) == bl
                            sg[:ns][mblk] = s_t[mblk] - bl * W
                        percore[c]["seg"].append(sg)

        sched.append(dict(ntiles=ntiles, tiles=tiles, pieces=pieces,
                          chunks=chunks, blocks=blocks, sb_sz=sb_sz))
        b0 += sb_sz

    for c in range(C):
        for k in ("idx", "seg", "wgt"):
            percore[c][k] = np.concatenate(percore[c][k])
    return sched, percore, (n2c, n2s, pid_tab)


def pack_gather_idx(flat_idx):
    """[n] int16 -> [128, n/16] buffer (16-partition wrap, replicated x8)."""
    n = len(flat_idx)
    assert n % 16 == 0
    b = flat_idx.reshape(n // 16, 16).T
    return np.tile(b, (8, 1)).copy()


def pack_tilewise(flat, ntiles):
    """[n=ntiles*128] -> [128, ntiles] (edge i -> part i%128, free i//128)."""
    return flat.reshape(ntiles, W).T.copy()


F32 = mybir.dt.float32
BF16 = mybir.dt.bfloat16
AF = mybir.ActivationFunctionType
ALU = mybir.AluOpType
W = 128
SLOPE = 0.15

# all small constants ship as two packed tensors (one bf16, one f32 that also
# carries seg/wgt) so startup is 2 big DMAs instead of ~24 serialized HWDGE ops
WPACK_BF = ([("iota", 128, 128), ("iden", 128, 128),
             ("W1l", 128, 512), ("W1r", 128, 512)]
            + [(f"W2l{k}", 128, 256) for k in range(4)]
            + [(f"W2r{k}", 128, 256) for k in range(4)]
            + [(f"W3l{k}", 128, 64) for k in range(2)]
            + [(f"W3r{k}", 128, 64) for k in range(2)]
            + [("Wpf", 64, 32), ("Wf2", 32, 2)])
WPACK_F32 = [("b1c", 128, 4), ("b2c", 128, 2), ("b3c", 64, 1),
             ("bpf1c", 32, 1), ("bf2c", 2, 1)]


def pack_offsets(spec):
    offs, o = {}, 0
    for name, part, cols in spec:
        offs[name] = (o, part, cols)
        o += cols
    return offs, o


BF_OFFS, BF_COLS = pack_offsets(WPACK_BF)
F32_OFFS, F32_COLS = pack_offsets(WPACK_F32)


def ceil_div(a, b):
    return (a + b - 1) // b


def build_kernel(cfg, sched, timing_mode=False):
    """cfg: dict(n_cores, n_nodes, slice, slice_pad, sb_sizes, ...)
    sched: from build_structure (list of superblock dicts)."""
    C = cfg["n_cores"]
    SP = cfg["slice_pad"]
    HI_BASE = cfg["hi_base"]
    N_LO_SB = cfg["n_lo_sb"]
    PIDN = cfg["pid_n"]
    NTL = sum(len(s["tiles"]) for s in sched)      # total gather tiles
    NPC = sum(len(bls) for s in sched
              for bls in s["pieces"].values())      # total (tile, block) pieces
    NID = NTL * W                                   # total edge slots
    MAXSBC = max(s["sb_sz"] for s in sched) * W
    sb_rows = [C * s["sb_sz"] * W for s in sched]   # pid rows per superblock
    sb_row0 = np.cumsum([0] + sb_rows).tolist()
    HI_ROWS = PIDN - HI_BASE

    def sb_chunks(sbc):
        return [(i * 512, min(512, sbc - i * 512)) for i in range(ceil_div(sbc, 512))]

    nc = bacc.Bacc("TRN2", target_bir_lowering=False, debug=False,
                   enable_asserts=True, num_devices=(1 if timing_mode else C))

    # ---------------- DRAM I/O ----------------
    xpid_d = nc.dram_tensor("xpid", [PIDN, 128], BF16, kind="ExternalInput")
    xT_d = nc.dram_tensor("xT", [128, SP], BF16, kind="ExternalInput")
    idx_d = nc.dram_tensor("idx", [128, NID // 16], mybir.dt.int16, kind="ExternalInput")
    wb_d = nc.dram_tensor("wb", [128, BF_COLS], BF16, kind="ExternalInput")
    wf_d = nc.dram_tensor("wf", [128, NPC + NTL + F32_COLS], F32, kind="ExternalInput")
    out_d = nc.dram_tensor("out", [2, SP], F32, kind="ExternalOutput")

    with tile.TileContext(nc) as tc:
        with (
            tc.tile_pool(name="const", bufs=1) as cp,
            tc.tile_pool(name="big", bufs=1) as bp,       # long-lived buffers
            tc.tile_pool(name="gath", bufs=GATHER_BUFS) as gp,      # gather chunks
            tc.tile_pool(name="work", bufs=16) as wp,      # onehot / drains / stages
            tc.tile_pool(name="psA", bufs=1, space="PSUM") as psA,   # agg psums
            tc.tile_pool(name="psB", bufs=2, space="PSUM") as psB,   # dense psums
            tc.tile_pool(name="dram", bufs=1, space="DRAM") as dp,
        ):
            # ---------------- constants ----------------
            wf_t = cp.tile([128, NPC + NTL + F32_COLS], F32, name="wf_t")
            nc.sync.dma_start(wf_t[:], wf_d.ap())
            wb_t = cp.tile([128, BF_COLS], BF16, name="wb_t")
            nc.sync.dma_start(wb_t[:], wb_d.ap())

            def w(name, c0=None, c1=None):
                o, p, c = BF_OFFS[name]
                if c0 is None:
                    c0, c1 = 0, c
                return wb_t[0:p, o + c0:o + c1]

            def wf(name, c0=None, c1=None):
                o, p, c = F32_OFFS[name]
                if c0 is None:
                    c0, c1 = 0, c
                return wf_t[0:p, NPC + NTL + o + c0:NPC + NTL + o + c1]

            # persistent self terms (SBUF-resident)
            selfb2 = [cp.tile([128, SP], BF16, name=f"selfb2_{m}") for m in range(2)]
            selfb3 = cp.tile([64, SP], BF16)
            # final output staged in SBUF, written to DRAM once at the end
            outsb = cp.tile([2, SP], F32, name="outsb")

            # DRAM scratch: allgathered tables, split lo/hi so gathers from
            # the lo pids only depend on the lo superblocks' collectives
            t2_lo = dp.tile([HI_BASE, 256], BF16)
            t2_hi = dp.tile([HI_ROWS, 256], BF16)
            t3_lo = dp.tile([HI_BASE, 128], BF16)
            t3_hi = dp.tile([HI_ROWS, 128], BF16)
            t2_slice = dp.tile([SP, 256], BF16)
            t3_slice = dp.tile([SP, 128], BF16)

            def emit_ag(si, slice_t, lo_t, hi_t):
                if timing_mode:
                    return
                col0 = sched[si]["blocks"][0] * W
                sbc = sched[si]["sb_sz"] * W
                r0 = sb_row0[si]
                dst_t, doff = (lo_t, 0) if si < N_LO_SB else (hi_t, HI_BASE)
                nc.gpsimd.collective_compute(
                    "AllGather", ALU.bypass, replica_groups=[list(range(C))],
                    ins=[slice_t[col0:col0 + sbc, :].opt()],
                    outs=[dst_t[r0 - doff:r0 - doff + C * sbc, :].opt()])



            def leaky(dst_ap, src_ap, bias=0.0):
                # Prelu honors alpha on HW (Lrelu ignores it: fixed slope)
                nc.scalar.activation(dst_ap, src_ap, AF.Prelu, bias=bias,
                                     alpha=SLOPE)

            # round-robin gather-buffer byte extents (see agg_phase)
            gbuf_extent = [0] * GATHER_BUFS
            gchunk_ctr = [0]

            # =========== generic aggregation phase ===========
            def agg_phase(d, row_elems, lo_ap, hi_ap, idx_t, drain_fn,
                          after_sb, tag, self_rhs=None):
                """d: used dims; row_elems: table row width.
                drain_fn(si, mp, ps_ap, col0, ncols): consume psum [dmp, ncols]
                holding agg_T rows [mp*128, ...) cols [col0, col0+ncols).
                after_sb(si, col0, sbc): emit dependent work for the sb."""
                n_mp = ceil_div(d, 128)
                dmp = min(d, 128)
                toff = 0
                poff = 0
                for si, s in enumerate(sched):
                    sbc = s["sb_sz"] * W
                    tiles = s["tiles"]
                    npieces = sum(len(s["pieces"][ht]) for ht in tiles)
                    ps = [psA.tile([dmp, sbc], F32, name=f"ps{tag}{mp}",
                                   tag=f"aggps{mp}", bufs=2)
                          for mp in range(n_mp)]
                    pctr = 0
                    for (h, t0, t1, nidx) in s["chunks"]:
                        nt = t1 - t0
                        g = gp.tile([128, nt, row_elems], BF16, name=f"g{tag}", tag="g")
                        base = lo_ap if h == 0 else hi_ap
                        # slots beyond the trimmed num_idxs still feed the
                        # matmul (masked by onehot zeros), so they must hold
                        # finite values (NaN x 0 = NaN in the PE). Stale bytes
                        # from a previous chunk in this buffer are fine; only
                        # a first use at a new byte extent could expose
                        # uninitialized SBUF -- fetch those untrimmed.
                        bi = gchunk_ctr[0] % GATHER_BUFS
                        ext = nt * row_elems
                        if nidx < nt * W and ext > gbuf_extent[bi]:
                            nidx = nt * W
                        gbuf_extent[bi] = max(gbuf_extent[bi], ext)
                        gchunk_ctr[0] += 1
                        nc.gpsimd.dma_gather(
                            g[:, :nt, :], base,
                            idx_t[:, (toff + t0) * 8:(toff + t1) * 8],
                            nidx, nidx, elem_size=row_elems, single_packet=False)
                        for t in range(t0, t1):
                            gt = toff + t
                            for bl in s["pieces"][tiles[t]]:
                                oh = wp.tile([128, W], BF16, name=f"oh{tag}", tag="oh", bufs=16)
                                gp_i = poff + pctr
                                nc.vector.tensor_scalar(
                                    oh[:], w("iota"), wf_t[:, gp_i:gp_i + 1],
                                    wf_t[:, NPC + gt:NPC + gt + 1],
                                    ALU.is_equal, ALU.mult)
                                for mp in range(n_mp):
                                    nc.tensor.matmul(
                                        ps[mp][:, bl * W:(bl + 1) * W],
                                        g[:, t - t0, mp * dmp:(mp + 1) * dmp],
                                        oh[:],
                                        start=(pctr == 0),
                                        stop=(pctr == npieces - 1
                                              and self_rhs is None))
                                pctr += 1
                    assert pctr == npieces
                    col0 = s["blocks"][0] * W
                    if self_rhs is not None:
                        # fold the self term into the psum on the PE (saves a
                        # DVE add + an engine hop on every drain chain)
                        io_, _, _ = BF_OFFS["iden"]
                        for mp in range(n_mp):
                            nc.tensor.matmul(
                                ps[mp][:, :sbc],
                                wb_t[0:dmp, io_:io_ + dmp],
                                self_rhs(mp)[0:dmp, col0:col0 + sbc],
                                start=False, stop=True)
                    # drain the single-buffered mps first: the next
                    # superblock's matmuls wait on their psum buffers
                    for mp in reversed(range(n_mp)):
                        cc = 0
                        while cc < sbc:
                            n = min(512, sbc - cc)
                            drain_fn(si, mp, ps[mp][:, cc:cc + n], col0 + cc, n)
                            cc += n
                    if after_sb is not None:
                        after_sb(si, col0, sbc)
                    toff += len(tiles)
                    poff += npieces

            # =========== L1: agg + interleaved dense ===========
            # only the first superblock's indices load upfront; the rest (and
            # most of xT) load behind the first superblock's gathers so the
            # DMA engines start useful work immediately
            idx_t = bp.tile([128, NID // 16], mybir.dt.int16, name="idx_t", tag="big1")
            NT0 = len(sched[0]["tiles"])
            SBC0 = sched[0]["sb_sz"] * W
            nc.sync.dma_start(idx_t[:, :NT0 * 8], idx_d.ap()[:, :NT0 * 8])
            xT_t = bp.tile([128, SP], BF16, name="xT_t", tag="big3")
            nc.sync.dma_start(xT_t[:, :SBC0], xT_d.ap()[:, :SBC0])

            agg1_sb = {}

            def drain1(si, mp, ps_ap, c0, n):
                if si not in agg1_sb:
                    agg1_sb[si] = (bp.tile([128, MAXSBC], BF16, name=f"agg1_{si}",
                                           tag="agg1", bufs=3),
                                   sched[si]["blocks"][0] * W)
                t_, col0 = agg1_sb[si]
                nc.scalar.activation(t_[:, c0 - col0:c0 - col0 + n], ps_ap, AF.Copy)

            def l1_dense(si, col0, sbc):
                if si == 0:
                    nc.sync.dma_start(idx_t[:, NT0 * 8:], idx_d.ap()[:, NT0 * 8:])
                    nc.sync.dma_start(xT_t[:, SBC0:], xT_d.ap()[:, SBC0:])
                a1, _ = agg1_sb.pop(si)
                for (off, n) in sb_chunks(sbc):
                    c0 = col0 + off
                    h1 = [wp.tile([128, 512], BF16, name=f"h1_{m}", tag=f"h1_{m}",
                                  bufs=2) for m in range(4)]
                    for m in range(4):
                        ph = psB.tile([128, 512], F32, name="ph1", tag="dense")
                        nc.tensor.matmul(ph[:, :n], w("W1l", m * 128, (m + 1) * 128),
                                         a1[:, off:off + n], start=True, stop=False)
                        nc.tensor.matmul(ph[:, :n], w("W1r", m * 128, (m + 1) * 128),
                                         xT_t[:, c0:c0 + n], start=False, stop=True)
                        leaky(h1[m][:, :n], ph[:, :n], bias=wf("b1c", m, m + 1))
                    for sub in range(n // 128):
                        pt = psB.tile([128, 256], F32, name="pt2", tag="dense")
                        for k in range(4):
                            nc.tensor.matmul(pt[:], h1[k][:, sub * 128:(sub + 1) * 128],
                                             w(f"W2l{k}"),
                                             start=(k == 0), stop=(k == 3))
                        st = wp.tile([128, 256], BF16, name="st2", tag="st2")
                        nc.scalar.activation(st[:], pt[:], AF.Copy)
                        r0 = c0 + sub * 128
                        nc.sync.dma_start(t2_slice[r0:r0 + 128, :], st[:])
                    for m in range(2):
                        pss = psB.tile([128, 512], F32, name="ps2", tag="dense")
                        for k in range(4):
                            nc.tensor.matmul(pss[:, :n],
                                             w(f"W2r{k}", m * 128, (m + 1) * 128),
                                             h1[k][:, :n], start=(k == 0), stop=(k == 3))
                        nc.scalar.activation(selfb2[m][:, c0:c0 + n], pss[:, :n],
                                             AF.Identity, bias=wf("b2c", m, m + 1))
                emit_ag(si, t2_slice, t2_lo, t2_hi)

            agg_phase(128, 128, xpid_d.ap(), xpid_d.ap()[HI_BASE:, :],
                      idx_t, drain1, l1_dense, "1")

            # =========== L2 ===========
            h2_sb = {}

            def drain2(si, mp, ps_ap, c0, n):
                key = (si, mp)
                if key not in h2_sb:
                    h2_sb[key] = (bp.tile([128, MAXSBC], BF16, name=f"h2_{si}_{mp}",
                                          tag=f"h2m{mp}", bufs=2),
                                  sched[si]["blocks"][0] * W)
                t_, col0 = h2_sb[key]
                leaky(t_[:, c0 - col0:c0 - col0 + n], ps_ap)

            def l2_dense(si, col0, sbc):
                h2m = [h2_sb.pop((si, mp))[0] for mp in range(2)]
                for sub in range(sbc // 128):
                    pt = psB.tile([128, 64], F32, name="pt3", tag="dense")
                    s0 = col0 + sub * 128
                    for k in range(2):
                        nc.tensor.matmul(pt[:], h2m[k][:, sub * 128:(sub + 1) * 128],
                                         w(f"W3l{k}"),
                                         start=(k == 0), stop=(k == 1))
                    st = wp.tile([128, 64], BF16, name="st3", tag="st3")
                    nc.scalar.activation(st[:], pt[:], AF.Copy)
                    nc.sync.dma_start(t3_slice[s0:s0 + 128, :64], st[:])
                for (off, n) in sb_chunks(sbc):
                    c0 = col0 + off
                    pss = psB.tile([64, 512], F32, name="ps3", tag="dense")
                    for k in range(2):
                        nc.tensor.matmul(pss[:, :n], w(f"W3r{k}"),
                                         h2m[k][:, off:off + n],
                                         start=(k == 0), stop=(k == 1))
                    nc.scalar.activation(selfb3[:, c0:c0 + n], pss[:, :n],
                                         AF.Identity, bias=wf("b3c"))
                emit_ag(si, t3_slice, t3_lo, t3_hi)

            agg_phase(256, 256, t2_lo[:], t2_hi[:], idx_t,
                      drain2, l2_dense, "2",
                      self_rhs=lambda mp: selfb2[mp])

            # =========== L3 + head ===========
            h3_sb = {}

            def drain3(si, mp, ps_ap, c0, n):
                if si not in h3_sb:
                    h3_sb[si] = (bp.tile([64, MAXSBC], BF16, name=f"h3_{si}",
                                         tag="h3", bufs=2),
                                 sched[si]["blocks"][0] * W)
                t_, col0 = h3_sb[si]
                leaky(t_[:, c0 - col0:c0 - col0 + n], ps_ap)

            def head(si, col0, sbc):
                h3t, _ = h3_sb.pop(si)
                for (off, n) in sb_chunks(sbc):
                    c0 = col0 + off
                    # pre_fc has no activation, so Wp/bp fuse into Wf1/bf1
                    # host-side: f1 = leaky(h3 @ (Wp Wf1) + (bp Wf1 + bf1))
                    pp = psB.tile([32, 512], F32, name="pp", tag="dense")
                    nc.tensor.matmul(pp[:, :n], w("Wpf"), h3t[:, off:off + n],
                                     start=True, stop=True)
                    f1 = wp.tile([32, 512], BF16, name="f1", tag="f1", bufs=2)
                    leaky(f1[:, :n], pp[:, :n], bias=wf("bpf1c"))

                    po = psB.tile([2, 512], F32, name="po", tag="dense")
                    nc.tensor.matmul(po[:, :n], w("Wf2"), f1[:, :n],
                                     start=True, stop=True)
                    nc.scalar.activation(outsb[:, c0:c0 + n], po[:, :n],
                                         AF.Identity, bias=wf("bf2c"))

            agg_phase(64, 128, t3_lo[:], t3_hi[:], idx_t,
                      drain3, head, "3",
                      self_rhs=lambda mp: selfb3)

            nc.sync.dma_start(out_d.ap()[:], outsb[:])

    nc.compile()
    return nc


def make_core_inputs(cfg, sched, percore, perm, inp):
    """Build per-core in_maps from the problem inputs dict."""
    C = cfg["n_cores"]
    SP = cfg["slice_pad"]
    SL = cfg["slice"]
    NN = cfg["n_nodes"]
    n2c, n2s, pid_tab = perm
    BFNP = mybir.dt.np(BF16)
    x = np.asarray(inp["x"], np.float32)
    iota = np.tile(np.arange(W, dtype=np.float32), (128, 1))

    xpid = np.zeros((cfg["pid_n"], 128), np.float32)
    xpid[pid_tab] = x

    # packed bf16 constants
    wvals_bf = dict(
        iota=iota,
        W1l=np.asarray(inp["W1l"], np.float32),
        W1r=np.asarray(inp["W1r"], np.float32),
        Wpf=np.asarray(inp["Wp"], np.float32) @ np.asarray(inp["Wf1"], np.float32),
        Wf2=np.asarray(inp["Wf2"], np.float32),
        iden=np.eye(128, dtype=np.float32),
    )
    for k in range(4):
        wvals_bf[f"W2l{k}"] = np.asarray(inp["W2l"], np.float32)[k * 128:(k + 1) * 128]
        wvals_bf[f"W2r{k}"] = np.asarray(inp["W2r"], np.float32)[k * 128:(k + 1) * 128]
    for k in range(2):
        wvals_bf[f"W3l{k}"] = np.asarray(inp["W3l"], np.float32)[k * 128:(k + 1) * 128]
        wvals_bf[f"W3r{k}"] = np.asarray(inp["W3r"], np.float32)[k * 128:(k + 1) * 128]
    wb = np.zeros((128, BF_COLS), np.float32)
    for name, (o, p, cc) in BF_OFFS.items():
        wb[0:p, o:o + cc] = wvals_bf[name]

    wvals_f32 = dict(
        b1c=np.asarray(inp["b1"], np.float32).reshape(4, 128).T,
        b2c=np.asarray(inp["b2"], np.float32).reshape(2, 128).T,
        b3c=np.asarray(inp["b3"], np.float32).reshape(64, 1),
        bpf1c=(np.asarray(inp["bp"], np.float32) @ np.asarray(inp["Wf1"], np.float32)
               + np.asarray(inp["bf1"], np.float32)).reshape(32, 1),
        bf2c=np.asarray(inp["bf2"], np.float32).reshape(2, 1),
    )

    shared = dict(wb=wb.astype(BFNP), xpid=xpid.astype(BFNP))
    in_maps = []
    for c in range(C):
        xs = np.zeros((128, SP), np.float32)
        mc = n2c == c
        xs[:, n2s[mc]] = x[mc].T
        npc = len(percore[c]["seg"]) // W
        ntl = len(percore[c]["wgt"]) // W
        wfp = np.zeros((128, npc + ntl + F32_COLS), np.float32)
        wfp[:, :npc] = pack_tilewise(percore[c]["seg"], npc)
        wfp[:, npc:npc + ntl] = pack_tilewise(percore[c]["wgt"], ntl)
        for name, (o, p, cc) in F32_OFFS.items():
            wfp[0:p, npc + ntl + o:npc + ntl + o + cc] = wvals_f32[name]
        m = dict(shared)
        m.update(
            xT=xs.astype(BFNP),
            idx=pack_gather_idx(percore[c]["idx"]),
            wf=wfp,
        )
        in_maps.append(m)
    return in_maps


# ----------------------------------------------------------------------
# public entry point
# ----------------------------------------------------------------------
_CACHE = {}


def _get_compiled(edge_index):
    key = hash(edge_index.tobytes())
    if key not in _CACHE:
        cfg = CFG_FULL
        sched, percore, perm = build_structure(edge_index, cfg)
        nc = build_kernel(cfg, sched)
        _CACHE[key] = (cfg, sched, percore, perm, nc)
    return _CACHE[key]


def _run(inputs, trace=False):
    inputs = {k: np.asarray(v) for k, v in inputs.items()}
    edge_index = np.asarray(inputs["edge_index"], np.int32)
    cfg, sched, percore, perm, nc = _get_compiled(edge_index)
    in_maps = make_core_inputs(cfg, sched, percore, perm, inputs)
    res = run_bass_kernel_spmd(nc, in_maps, core_ids=list(range(cfg["n_cores"])),
                               trace=trace)
    n2c, n2s, _ = perm
    stacked = np.stack([res.results[c]["out"] for c in range(cfg["n_cores"])])
    out = stacked[n2c, :, n2s].astype(np.float32)
    return out, res


def kernel(**inputs):
    out, _ = _run(inputs)
    return out

